# revision 1
# baseline (speedup 1.0000x reference)
"""Trainium2 Bass kernel for nn_MultiHeadAttention_41936060678770.

LinBERT-style linear attention:
  qh/kh/vh = LN(x) @ W + b  (per-stream LN, 16 heads x 64 dim)
  phi = elu(.)+1 ;  phi_k masked
  kv = sum_s phi_k (x) vh ; z = sum_s phi_k
  attn = (phi_q @ kv) / (phi_q @ z + eps)
  out = q + attn @ fc_w + fc_b

Sharding: 8 cores, tokens split 8-ways over flattened (B*S); each pair of
cores (2c, 2c+1) holds one batch, so the [16,64,65] kv/z state is
all-reduced within core pairs; everything else is fully local.

Layout strategy per core (2048 tokens, 16 tiles of 128):
  - activations live tokens-on-partitions ("natural"); contraction operands
    are produced by XBAR DMA-transpose of bf16 tiles (2-byte only, free of
    PE/DVE cost).
  - LN is folded to one fused DVE op: xn = (x - mu) * rsig  (g==1, b==0
    fast path verified on host; general path scales W rows by g on device
    and adds c = b@W + b_proj via a broadcast tile).
  - All big matmuls run in bf16 with fp32 PSUM accumulation.
  - kv state and z accumulate in PSUM across all 16 token tiles
    ([64,65] per head packed 8 heads/bank x 2 banks), then AllReduce.
"""
import sys

sys.path.insert(0, "/opt/trn_rl_repo")

import numpy as np

import concourse.bacc as bacc
import concourse.bass as bass
import concourse.tile as tile
import concourse.mybir as mybir
from concourse.bass_utils import run_bass_kernel_spmd

F32 = mybir.dt.float32
BF16 = mybir.dt.bfloat16
AF = mybir.ActivationFunctionType
ALU = mybir.AluOpType

B, S, HS = 4, 4096, 1024
NH, D = 16, 64
NCORES = 8
TOK = B * S // NCORES          # 2048 rows per core
NT = TOK // 128                # 16 token tiles
KT = HS // 128                 # 8 hidden tiles
LN_EPS = 1e-5
ATT_EPS = 1e-6


def _ln_project(nc, pool, psp, stat_pool, eps_t, x_nat, w_sb, c_bcast):
    """LN-center+scale -> transpose -> project. Returns list of 2 psum tiles
    [128,512] (the projected output chunks, pre-activation) plus the fused
    normalized-transposed tile so callers can keep references alive."""
    mv = stat_pool.tile([128, 2], F32, tag="mv")
    stats = stat_pool.tile([128, 2, 6], F32, tag="stats")
    nc.vector.bn_stats(out=stats[:, 0, :], in_=x_nat[:, 0:512])
    nc.vector.bn_stats(out=stats[:, 1, :], in_=x_nat[:, 512:1024])
    nc.vector.bn_aggr(out=mv[:], in_=stats[:])
    # rsig = rsqrt(var+eps) fully on DVE (quake seed + 2 Newton steps) —
    # ACT then only ever runs Exp/Copy (one table set, no ~1.3us
    # ACT_TABLE_LOAD thrash per LN<->elu switch).
    veps = stat_pool.tile([128, 1], F32, tag="veps")
    nc.vector.tensor_scalar_add(out=veps[:], in0=mv[:, 1:2], scalar1=LN_EPS)
    seed = stat_pool.tile([128, 1], mybir.dt.int32, tag="seed")
    nc.vector.tensor_scalar(
        out=seed[:], in0=veps[:].bitcast(mybir.dt.int32),
        scalar1=1, scalar2=None, op0=ALU.arith_shift_right)
    nc.vector.tensor_scalar(
        out=seed[:], in0=seed[:], scalar1=-1, scalar2=0x5F3759DF,
        op0=ALU.mult, op1=ALU.add)
    y0 = seed[:].bitcast(F32)
    t_nr = stat_pool.tile([128, 1], F32, tag="t_nr")
    sig = stat_pool.tile([128, 1], F32, tag="sig")
    nc.vector.tensor_tensor(out=t_nr[:], in0=y0, in1=y0, op=ALU.mult)
    nc.vector.tensor_tensor(out=t_nr[:], in0=t_nr[:], in1=veps[:],
                            op=ALU.mult)
    nc.vector.tensor_scalar(out=t_nr[:], in0=t_nr[:], scalar1=-0.5,
                            scalar2=1.5, op0=ALU.mult, op1=ALU.add)
    nc.vector.tensor_tensor(out=sig[:], in0=y0, in1=t_nr[:], op=ALU.mult)
    nc.vector.tensor_tensor(out=t_nr[:], in0=sig[:], in1=sig[:], op=ALU.mult)
    nc.vector.tensor_tensor(out=t_nr[:], in0=t_nr[:], in1=veps[:],
                            op=ALU.mult)
    nc.vector.tensor_scalar(out=t_nr[:], in0=t_nr[:], scalar1=-0.5,
                            scalar2=1.5, op0=ALU.mult, op1=ALU.add)
    nc.vector.tensor_tensor(out=sig[:], in0=sig[:], in1=t_nr[:], op=ALU.mult)
    xn = pool.tile([128, HS], BF16, tag="xn")
    nc.vector.tensor_scalar(
        out=xn[:], in0=x_nat[:], scalar1=mv[:, 0:1], scalar2=sig[:],
        op0=ALU.subtract, op1=ALU.mult,
    )
    xnT = pool.tile([128, KT, 128], BF16, tag="xnT")
    nc.sync.dma_start_transpose(out=xnT[:], in_=xn[:])

    ps_chunks = []
    for c in range(2):
        ps = psp.tile([128, 512], F32, tag="proj")
        for kt in range(KT):
            nc.tensor.matmul(
                ps[:], xnT[:, kt, :], w_sb[:, kt, c * 512:(c + 1) * 512],
                start=(kt == 0), stop=(kt == KT - 1),
            )
        if c_bcast is not None:
            nc.vector.tensor_tensor(
                out=ps[:], in0=ps[:], in1=c_bcast[:, c * 512:(c + 1) * 512],
                op=ALU.add,
            )
        ps_chunks.append(ps)
    return ps_chunks, xnT


def _elu1(nc, pool, src, out_ap, mask_col):
    """out = elu(src)+1 = exp(min(src,0)) + max(src,0), optionally * mask."""
    tmin = pool.tile([128, 512], F32, tag="tmin")
    nc.vector.tensor_scalar_min(out=tmin[:], in0=src[:], scalar1=0.0)
    texp = pool.tile([128, 512], F32, tag="texp")
    nc.scalar.activation(out=texp[:], in_=tmin[:], func=AF.Exp)
    if mask_col is None:
        nc.vector.scalar_tensor_tensor(
            out=out_ap, in0=src[:], scalar=0.0, in1=texp[:],
            op0=ALU.max, op1=ALU.add,
        )
    else:
        tphi = pool.tile([128, 512], F32, tag="tphi")
        nc.vector.scalar_tensor_tensor(
            out=tphi[:], in0=src[:], scalar=0.0, in1=texp[:],
            op0=ALU.max, op1=ALU.add,
        )
        nc.vector.tensor_scalar_mul(out=out_ap, in0=tphi[:], scalar1=mask_col)


def build(g_trivial: bool, c_trivial: bool, mask_trivial: bool,
          replica_groups, _skip_collective=False):
    nc = bacc.Bacc(None)

    qx_d = nc.dram_tensor("qx", [TOK, HS], F32, kind="ExternalInput")
    kx_d = nc.dram_tensor("kx", [TOK, HS], F32, kind="ExternalInput")
    vx_d = nc.dram_tensor("vx", [TOK, HS], F32, kind="ExternalInput")
    w_d = {
        "q": nc.dram_tensor("wq", [HS, HS], F32, kind="ExternalInput"),
        "k": nc.dram_tensor("wk", [HS, HS], F32, kind="ExternalInput"),
        "v": nc.dram_tensor("wv", [HS, HS], F32, kind="ExternalInput"),
        "fc": nc.dram_tensor("fcw", [HS, HS], F32, kind="ExternalInput"),
    }
    if not mask_trivial:
        mask_d = nc.dram_tensor("maskx", [TOK, 1], F32, kind="ExternalInput")
    if not g_trivial:
        g_d = {s: nc.dram_tensor(f"g_{s}", [HS], F32, kind="ExternalInput")
               for s in ("q", "k", "v")}
    if not c_trivial:
        # c vectors are computed on host?? no: computed on device from b/bias
        b_d = {s: nc.dram_tensor(f"b_{s}", [HS], F32, kind="ExternalInput")
               for s in ("q", "k", "v")}
        pb_d = {s: nc.dram_tensor(f"pb_{s}", [HS], F32, kind="ExternalInput")
                for s in ("q", "k", "v")}
        fcb_d = nc.dram_tensor("fcb", [HS], F32, kind="ExternalInput")

    out_d = nc.dram_tensor("out", [TOK, HS], F32, kind="ExternalOutput")

    from contextlib import ExitStack
    with tile.TileContext(nc) as tc, ExitStack() as ctx:
        wpool = ctx.enter_context(tc.tile_pool(name="weights", bufs=1))
        consts = ctx.enter_context(tc.tile_pool(name="consts", bufs=1))
        proj_ps = ctx.enter_context(
            tc.tile_pool(name="proj_ps", bufs=4, space="PSUM"))
        dram_p = ctx.enter_context(
            tc.tile_pool(name="dram", bufs=1, space="DRAM"))

        eps_t = consts.tile([128, 1], F32)
        nc.vector.memset(eps_t[:], LN_EPS)
        att_eps_t = consts.tile([128, 1], F32)
        nc.vector.memset(att_eps_t[:], ATT_EPS)

        # ---------------- weights ----------------
        w_sb = {}
        for s in ("q", "k", "v", "fc"):
            w_sb[s] = wpool.tile([128, KT, HS], BF16, tag=f"w_{s}", name=f"w_{s}")
            src = w_d[s].rearrange("(kt p) n -> p kt n", p=128)
            if g_trivial or s == "fc":
                nc.gpsimd.dma_start(out=w_sb[s][:], in_=src)
            else:
                wtmp = consts.tile([128, KT, HS], F32, tag="wtmp")
                nc.sync.dma_start(out=wtmp[:], in_=src)
                g_sb = consts.tile([128, KT], F32, tag=f"gsb_{s}")
                nc.sync.dma_start(
                    out=g_sb[:], in_=g_d[s].rearrange("(kt p) -> p kt", p=128))
                for kt in range(KT):
                    nc.vector.tensor_scalar_mul(
                        out=w_sb[s][:, kt, :], in0=wtmp[:, kt, :],
                        scalar1=g_sb[:, kt:kt + 1])

        # c = b @ (g*W) + proj_bias, broadcast across partitions
        c_bc = {"q": None, "k": None, "v": None}
        fcb_bc = None
        if not c_trivial:
            for s in ("q", "k", "v"):
                b_bf = consts.tile([128, KT], BF16, tag=f"bbf_{s}")
                nc.gpsimd.dma_start(
                    out=b_bf[:], in_=b_d[s].rearrange("(kt p) -> p kt", p=128))
                cps = proj_ps.tile([1, HS], F32, tag="c_ps")
                for kt in range(KT):
                    nc.tensor.matmul(cps[:], b_bf[:, kt:kt + 1],
                                     w_sb[s][:, kt, :],
                                     start=(kt == 0), stop=(kt == KT - 1))
                crow = consts.tile([1, HS], F32, tag=f"crow_{s}")
                pbrow = consts.tile([1, HS], F32, tag=f"pbrow_{s}")
                nc.sync.dma_start(out=pbrow[:], in_=pb_d[s][None, :])
                nc.vector.tensor_tensor(out=crow[:], in0=cps[:], in1=pbrow[:],
                                        op=ALU.add)
                c_bc[s] = consts.tile([128, HS], F32, tag=f"cbc_{s}", name=f"cbc_{s}")
                nc.gpsimd.partition_broadcast(c_bc[s][:], crow[:])
            fcb_row = consts.tile([1, HS], F32, tag="fcb_row")
            nc.sync.dma_start(out=fcb_row[:], in_=fcb_d[None, :])
            fcb_bc = consts.tile([128, HS], F32, tag="fcb_bc")
            nc.gpsimd.partition_broadcast(fcb_bc[:], fcb_row[:])

        # ---------------- sweep 1: K/V + kv state ----------------
        kv_sb = consts.tile([128, 8, D + 1], F32, tag="kv_sb")
        with (
            tc.tile_pool(name="kv_ps", bufs=1, space="PSUM") as kv_psp,
            tc.tile_pool(name="s1", bufs=4) as s1,
            tc.tile_pool(name="stat1", bufs=8) as stat1,
        ):
            kv_ps = [kv_psp.tile([128, 4, D + 1], F32, tag=f"kv{b}", name=f"kv{b}",
                                 padded_shape=[128, 4, 128])
                     for b in range(2)]
            for i in range(NT):
                r0 = i * 128
                k_nat = s1.tile([128, HS], BF16, tag="k_nat")
                nc.gpsimd.dma_start(out=k_nat[:], in_=kx_d[r0:r0 + 128, :])
                v_nat = s1.tile([128, HS], BF16, tag="v_nat")
                nc.gpsimd.dma_start(out=v_nat[:], in_=vx_d[r0:r0 + 128, :])
                mask_col = None
                if not mask_trivial:
                    mcol = stat1.tile([128, 1], F32, tag="mcol")
                    nc.sync.dma_start(out=mcol[:], in_=mask_d[r0:r0 + 128, :])
                    mask_col = mcol[:]

                kh_ps, _knT = _ln_project(nc, s1, proj_ps, stat1, eps_t,
                                          k_nat, w_sb["k"], c_bc["k"])
                phi_k = s1.tile([128, HS], BF16, tag="phi_k")
                for c in range(2):
                    _elu1(nc, s1, kh_ps[c], phi_k[:, c * 512:(c + 1) * 512],
                          mask_col)

                vh_ps, _vnT = _ln_project(nc, s1, proj_ps, stat1, eps_t,
                                          v_nat, w_sb["v"], c_bc["v"])
                vh_aug = s1.tile([128, NH, D + 1], BF16, tag="vh_aug")
                nc.vector.memset(vh_aug[:, :, D:D + 1], 1.0)
                for c in range(2):
                    nc.vector.tensor_copy(
                        out=vh_aug[:, c * 8:(c + 1) * 8, 0:D],
                        in_=vh_ps[c][:].rearrange("p (n d) -> p n d", d=D))

                for n in range(NH):
                    beta, j, hs = n // 8, (n // 2) % 4, (n % 2) * 64
                    nc.tensor.matmul(
                        kv_ps[beta][hs:hs + 64, j, :],
                        phi_k[:, n * D:(n + 1) * D],
                        vh_aug[:, n, :],
                        start=(i == 0), stop=(i == NT - 1),
                        tile_position=(0, hs),
                        skip_group_check=True,
                    )

            nc.vector.tensor_copy(out=kv_sb[:, 0:4, :], in_=kv_ps[0][:])
            nc.vector.tensor_copy(out=kv_sb[:, 4:8, :], in_=kv_ps[1][:])

        # ---------------- all-reduce kv state within batch pairs ----------
        # kv2 holds the reduced state as 8 block-diagonal [128, 130] bf16
        # operands (head-pair 2m/2m+1), so the num/den matmul is a plain
        # K=128 matmul at base partition 0 (operand base_partition=64
        # matmuls fault the exec unit on hardware).
        kv2 = consts.tile([128, 8, 2 * (D + 1)], BF16, tag="kv2")
        nc.vector.memset(kv2[:], 0.0)
        if _skip_collective:
            nc.vector.tensor_copy(out=kv2[0:64, :, 0:D + 1],
                                  in_=kv_sb[0:64, :, :])
            nc.vector.tensor_copy(out=kv2[64:128, :, D + 1:2 * (D + 1)],
                                  in_=kv_sb[64:128, :, :])
        else:
            cc_in = dram_p.tile([128, 8, D + 1], F32)
            cc_out = dram_p.tile([128, 8, D + 1], F32)
            nc.gpsimd.dma_start(out=cc_in[:], in_=kv_sb[:])
            nc.gpsimd.collective_compute(
                "AllReduce", ALU.add, replica_groups=replica_groups,
                ins=[cc_in.opt()], outs=[cc_out.opt()],
            )
            nc.gpsimd.dma_start(out=kv2[0:64, :, 0:D + 1],
                                in_=cc_out[0:64, :, :])
            nc.gpsimd.dma_start(out=kv2[64:128, :, D + 1:2 * (D + 1)],
                                in_=cc_out[64:128, :, :])

        # ---------------- sweep 2: Q -> attn -> fc -> out ----------------
        with (
            tc.tile_pool(name="nd_ps", bufs=4, space="PSUM") as nd_psp,
            tc.tile_pool(name="s2", bufs=4) as s2,
            tc.tile_pool(name="stat2", bufs=8) as stat2,
        ):
            for i in range(NT):
                r0 = i * 128
                q_nat = s2.tile([128, HS], F32, tag="q_nat")
                nc.gpsimd.dma_start(out=q_nat[:], in_=qx_d[r0:r0 + 128, :])

                qh_ps, _qnT = _ln_project(nc, s2, proj_ps, stat2, eps_t,
                                          q_nat, w_sb["q"], c_bc["q"])
                phi_q = s2.tile([128, HS], BF16, tag="phi_q")
                for c in range(2):
                    _elu1(nc, s2, qh_ps[c], phi_q[:, c * 512:(c + 1) * 512],
                          None)
                phi_qT = s2.tile([128, KT, 128], BF16, tag="phi_qT")
                nc.sync.dma_start_transpose(out=phi_qT[:], in_=phi_q[:])

                attn = s2.tile([128, HS], BF16, tag="attn")
                # 2 head-pairs per PSUM bank; den processed batched per tile
                nds = []
                den = stat2.tile([128, NH], F32, tag="den")
                for m in range(8):       # head pair (2m, 2m+1) per matmul
                    if m % 2 == 0:
                        nd2 = nd_psp.tile([128, 2, 2 * (D + 1)], F32,
                                          tag="nd", name="nd",
                                          padded_shape=[128, 2, 256])
                        nds.append(nd2)
                    nd = nd2[:, m % 2, :]
                    nc.tensor.matmul(
                        nd, phi_qT[:, m, :], kv2[:, m, :],
                        start=True, stop=True,
                    )
                    nc.vector.tensor_copy(out=den[:, 2 * m:2 * m + 2],
                                          in_=nd[:, D::D + 1])
                rd = stat2.tile([128, NH], F32, tag="rd")
                nc.vector.tensor_scalar_add(out=rd[:], in0=den[:],
                                            scalar1=ATT_EPS)
                nc.vector.reciprocal(out=rd[:], in_=rd[:])
                for n in range(NH):
                    nd = nds[n // 4][:, (n // 2) % 2, :]
                    nc.scalar.activation(
                        out=attn[:, n * D:(n + 1) * D],
                        in_=nd[:, (n % 2) * (D + 1):(n % 2) * (D + 1) + D],
                        func=AF.Copy, bias=0.0, scale=rd[:, n:n + 1])

                attnT = s2.tile([128, KT, 128], BF16, tag="attnT")
                nc.sync.dma_start_transpose(out=attnT[:], in_=attn[:])

                out_sb = s2.tile([128, HS], F32, tag="out_sb")
                for c in range(2):
                    ps = proj_ps.tile([128, 512], F32, tag="proj")
                    for kt in range(KT):
                        nc.tensor.matmul(
                            ps[:], attnT[:, kt, :],
                            w_sb["fc"][:, kt, c * 512:(c + 1) * 512],
                            start=(kt == 0), stop=(kt == KT - 1))
                    if fcb_bc is not None:
                        nc.vector.tensor_tensor(
                            out=ps[:], in0=ps[:],
                            in1=fcb_bc[:, c * 512:(c + 1) * 512], op=ALU.add)
                    nc.vector.tensor_tensor(
                        out=out_sb[:, c * 512:(c + 1) * 512], in0=ps[:],
                        in1=q_nat[:, c * 512:(c + 1) * 512], op=ALU.add)
                nc.gpsimd.dma_start(out=out_d[r0:r0 + 128, :], in_=out_sb[:])

    nc.compile()
    return nc


_BUILD_CACHE = {}


def _get_nc(flags, replica_groups):
    key = (flags, tuple(tuple(g) for g in replica_groups))
    if key not in _BUILD_CACHE:
        _BUILD_CACHE[key] = build(*flags, replica_groups)
    return _BUILD_CACHE[key]


def kernel(q, k, v, ln_q_g, ln_q_b, wq, bq, ln_k_g, ln_k_b, wk, bk,
           ln_v_g, ln_v_b, wv, bv, fc_w, fc_b, mask):
    q = np.ascontiguousarray(q, np.float32).reshape(B * S, HS)
    k = np.ascontiguousarray(k, np.float32).reshape(B * S, HS)
    v = np.ascontiguousarray(v, np.float32).reshape(B * S, HS)
    mask_f = np.ascontiguousarray(mask, np.float32).reshape(B * S, 1)
    wq = np.ascontiguousarray(wq, np.float32)
    wk = np.ascontiguousarray(wk, np.float32)
    wv = np.ascontiguousarray(wv, np.float32)
    fc_w = np.ascontiguousarray(fc_w, np.float32)

    g_trivial = all(np.all(x == 1.0) for x in (ln_q_g, ln_k_g, ln_v_g))
    c_trivial = all(np.all(x == 0.0) for x in
                    (ln_q_b, ln_k_b, ln_v_b, bq, bk, bv, fc_b))
    mask_trivial = bool(np.all(mask_f == 1.0))

    groups = [[0, 1], [2, 3], [4, 5], [6, 7]]
    nc = _get_nc((g_trivial, c_trivial, mask_trivial), groups)

    in_maps = []
    for c in range(NCORES):
        r0, r1 = c * TOK, (c + 1) * TOK
        m = {
            "qx": q[r0:r1], "kx": k[r0:r1], "vx": v[r0:r1],
            "wq": wq, "wk": wk, "wv": wv, "fcw": fc_w,
        }
        if not mask_trivial:
            m["maskx"] = mask_f[r0:r1]
        if not g_trivial:
            m.update({"g_q": np.asarray(ln_q_g, np.float32),
                      "g_k": np.asarray(ln_k_g, np.float32),
                      "g_v": np.asarray(ln_v_g, np.float32)})
        if not c_trivial:
            m.update({"b_q": np.asarray(ln_q_b, np.float32),
                      "b_k": np.asarray(ln_k_b, np.float32),
                      "b_v": np.asarray(ln_v_b, np.float32),
                      "pb_q": np.asarray(bq, np.float32),
                      "pb_k": np.asarray(bk, np.float32),
                      "pb_v": np.asarray(bv, np.float32),
                      "fcb": np.asarray(fc_b, np.float32)})
        in_maps.append(m)

    res = run_bass_kernel_spmd(nc, in_maps, list(range(NCORES)))
    out = np.concatenate([res.results[c]["out"] for c in range(NCORES)], 0)
    return out.reshape(B, S, HS).astype(np.float32)



# revision 2
# speedup vs baseline: 1.0164x; 1.0164x over previous
"""Trainium2 Bass kernel v2 for nn_MultiHeadAttention_41936060678770.

LinBERT linear attention, 8 cores, tokens split 8-ways over (B*S); core
pairs (2c,2c+1) share a batch and AllReduce their kv state.

v2 design vs baseline:
- host converts all inputs to bf16 (halves HBM traffic); g folded into W
  on host; s = colsum(W) rows passed in.
- LN folded algebraically: LN(x)@W = rsig*(x@W - mu*(1'W)).  Projections
  run on RAW xT (DMA-transposed straight from DRAM, no on-chip xn
  transpose); correction u = ps - mu (x) s via one scalar_tensor_tensor
  with a partition-broadcast s tile; rsig fused into the elu decomposition
  tensor_scalar ops.  No bn_stats (tensor_reduce + tensor_tensor_reduce).
- kv state pair-packed: 8 matmuls of [128,130] per 128-token tile
  (block-diagonal, off-block cross terms land in never-read slots).
- 3 phases: A (K/V + kv state, 256-token macro tiles), collective kicked,
  Q (projections + phi_q -> transpose -> stash; hides the collective),
  B (num/den pair matmuls, broadcast-AP rd scale on evac, attn xbar, fc,
  residual from a bf16 q reload, bf16 output).
"""
import sys

sys.path.insert(0, "/opt/trn_rl_repo")

import numpy as np
import ml_dtypes

import concourse.bacc as bacc
import concourse.bass as bass
import concourse.tile as tile
import concourse.mybir as mybir
from concourse.bass_utils import run_bass_kernel_spmd

F32 = mybir.dt.float32
BF16 = mybir.dt.bfloat16
AF = mybir.ActivationFunctionType
ALU = mybir.AluOpType
AXIS = mybir.AxisListType

B, S, HS = 4, 4096, 1024
NH, D = 16, 64
NCORES = 8
TOK = B * S // NCORES          # 2048 rows per core
NMACRO = 8                     # 256-token macro tiles in phases A/Q
NT = 16                        # 128-token tiles in phase B
LN_EPS = 1e-5
ATT_EPS = 1e-6
RH = 1.0 / HS

# exotic-op toggles (False = baseline-proven equivalent)
USE_REDUCE_STATS = False     # tensor_reduce/tensor_tensor_reduce vs bn_stats
USE_BCAST_EVAC = False       # to_broadcast rd scale vs per-head ACT copies
USE_STRIDED_MEMSET = False   # strided ones-col memset vs full memset
USE_4D_VH = False            # 4D strided vh write vs two 3D writes


def _newton_rsqrt(nc, pool, veps, ncols):
    """rsig = rsqrt(veps) on DVE: quake seed + 2 Newton steps. [128,ncols]."""
    seed = pool.tile([128, ncols], mybir.dt.int32, tag="nw_seed")
    nc.vector.tensor_scalar(
        out=seed[:], in0=veps[:].bitcast(mybir.dt.int32),
        scalar1=1, scalar2=None, op0=ALU.arith_shift_right)
    nc.vector.tensor_scalar(
        out=seed[:], in0=seed[:], scalar1=-1, scalar2=0x5F3759DF,
        op0=ALU.mult, op1=ALU.add)
    y0 = seed[:].bitcast(F32)
    t = pool.tile([128, ncols], F32, tag="nw_t")
    sig = pool.tile([128, ncols], F32, tag="nw_sig")
    nc.vector.tensor_tensor(out=t[:], in0=y0, in1=y0, op=ALU.mult)
    nc.vector.tensor_tensor(out=t[:], in0=t[:], in1=veps[:], op=ALU.mult)
    nc.vector.tensor_scalar(out=t[:], in0=t[:], scalar1=-0.5, scalar2=1.5,
                            op0=ALU.mult, op1=ALU.add)
    nc.vector.tensor_tensor(out=sig[:], in0=y0, in1=t[:], op=ALU.mult)
    nc.vector.tensor_tensor(out=t[:], in0=sig[:], in1=sig[:], op=ALU.mult)
    nc.vector.tensor_tensor(out=t[:], in0=t[:], in1=veps[:], op=ALU.mult)
    nc.vector.tensor_scalar(out=t[:], in0=t[:], scalar1=-0.5, scalar2=1.5,
                            op0=ALU.mult, op1=ALU.add)
    nc.vector.tensor_tensor(out=sig[:], in0=sig[:], in1=t[:], op=ALU.mult)
    return sig


def _stats(nc, pool, stat_pool, x_nat, sx, sx2, j):
    """sx[:,j] = sum(x), sx2[:,j] = sum(x^2) for x_nat [128,1024] bf16."""
    if USE_REDUCE_STATS:
        nc.vector.tensor_reduce(out=sx[:, j:j + 1], in_=x_nat,
                                axis=AXIS.X, op=ALU.add)
        scr = pool.tile([128, HS], BF16, tag="scr")
        nc.vector.tensor_tensor_reduce(
            out=scr[:], in0=x_nat, in1=x_nat, scale=1.0, scalar=0.0,
            op0=ALU.mult, op1=ALU.add, accum_out=sx2[:, j:j + 1])
    else:
        # bn_stats path: sx gets mean*H, sx2 gets (var+mu^2)*H so the
        # downstream mu/var algebra is unchanged.
        stats = stat_pool.tile([128, 2, 6], F32, tag="bn_st")
        mv = stat_pool.tile([128, 2], F32, tag="bn_mv")
        nc.vector.bn_stats(out=stats[:, 0, :], in_=x_nat[:, 0:512])
        nc.vector.bn_stats(out=stats[:, 1, :], in_=x_nat[:, 512:1024])
        nc.vector.bn_aggr(out=mv[:], in_=stats[:])
        nc.vector.tensor_scalar_mul(out=sx[:, j:j + 1], in0=mv[:, 0:1],
                                    scalar1=float(HS))
        # sx2/H = var + mu^2
        musq = stat_pool.tile([128, 1], F32, tag="bn_musq")
        nc.vector.tensor_tensor(out=musq[:], in0=mv[:, 0:1], in1=mv[:, 0:1],
                                op=ALU.mult)
        tmp = stat_pool.tile([128, 1], F32, tag="bn_tmp")
        nc.vector.tensor_tensor(out=tmp[:], in0=mv[:, 1:2], in1=musq[:],
                                op=ALU.add)
        nc.vector.tensor_scalar_mul(out=sx2[:, j:j + 1], in0=tmp[:],
                                    scalar1=float(HS))


def build(c_trivial: bool, mask_trivial: bool, replica_groups):
    nc = bacc.Bacc(None)

    qx_d = nc.dram_tensor("qx", [TOK, HS], BF16, kind="ExternalInput")
    kx_d = nc.dram_tensor("kx", [TOK, HS], BF16, kind="ExternalInput")
    vx_d = nc.dram_tensor("vx", [TOK, HS], BF16, kind="ExternalInput")
    w_d = {s: nc.dram_tensor(f"w_{s}", [HS, HS], BF16, kind="ExternalInput")
           for s in ("q", "k", "v", "fc")}
    s_d = {s: nc.dram_tensor(f"s_{s}", [1, HS], BF16, kind="ExternalInput")
           for s in ("q", "k", "v")}
    if not c_trivial:
        c_d = {s: nc.dram_tensor(f"c_{s}", [1, HS], F32, kind="ExternalInput")
               for s in ("q", "k", "v", "fc")}
    if not mask_trivial:
        mask_d = nc.dram_tensor("maskx", [TOK, 1], F32, kind="ExternalInput")

    out_d = nc.dram_tensor("out", [TOK, HS], BF16, kind="ExternalOutput")

    from contextlib import ExitStack
    with tile.TileContext(nc) as tc, ExitStack() as ctx:
        wpool = ctx.enter_context(tc.tile_pool(name="weights", bufs=1))
        consts = ctx.enter_context(tc.tile_pool(name="consts", bufs=1))
        dram_p = ctx.enter_context(
            tc.tile_pool(name="dram", bufs=1, space="DRAM"))

        # ---------------- weights + const rows ----------------
        w_sb = {}
        for s in ("k", "v", "q", "fc"):
            w_sb[s] = wpool.tile([128, 8, HS], BF16, tag=f"w_{s}",
                                 name=f"w_{s}")
            nc.gpsimd.dma_start(
                out=w_sb[s][:], in_=w_d[s].rearrange("(kt p) n -> p kt n",
                                                     p=128))
        s_bc = {}
        for s in ("k", "v", "q"):
            row = consts.tile([1, HS], BF16, tag=f"srow_{s}")
            nc.gpsimd.dma_start(out=row[:], in_=s_d[s][:, :])
            s_bc[s] = consts.tile([128, HS], BF16, tag=f"sbc_{s}",
                                  name=f"sbc_{s}")
            nc.gpsimd.partition_broadcast(s_bc[s][:], row[:])
        c_bc = {"q": None, "k": None, "v": None, "fc": None}
        if not c_trivial:
            for s in ("q", "k", "v", "fc"):
                row = consts.tile([1, HS], F32, tag=f"crow_{s}")
                nc.gpsimd.dma_start(out=row[:], in_=c_d[s][:, :])
                c_bc[s] = consts.tile([128, HS], F32, tag=f"cbc_{s}",
                                      name=f"cbc_{s}")
                nc.gpsimd.partition_broadcast(c_bc[s][:], row[:])

        kv_sb = consts.tile([128, 8, 2 * (D + 1)], F32, tag="kv_sb")

        def project_chunk(psp, xT_a, st, c):
            """Raw projection of one 128-token tile, output chunk c.
            xT_a: [128, 8, 128] transposed raw activations.
            Returns PSUM tile [128,512] = x @ W[:, c*512:(c+1)*512]."""
            ps = psp.tile([128, 512], F32, tag="proj")
            for kt in range(8):
                nc.tensor.matmul(
                    ps[:], xT_a[:, kt, :],
                    w_sb[st][:, kt, c * 512:(c + 1) * 512],
                    start=(kt == 0), stop=(kt == 7))
            return ps

        def ln_correct(pool, ps, st, c, negmu_col):
            """u = ps - mu (x) s  (+ c row), bf16 out."""
            u = pool.tile([128, 512], BF16, tag="u")
            nc.vector.scalar_tensor_tensor(
                out=u[:], in0=s_bc[st][:, c * 512:(c + 1) * 512],
                scalar=negmu_col, in1=ps[:], op0=ALU.mult, op1=ALU.add)
            if c_bc[st] is not None:
                nc.vector.tensor_tensor(
                    out=u[:], in0=u[:],
                    in1=c_bc[st][:, c * 512:(c + 1) * 512], op=ALU.add)
            return u

        def elu_phi(pool, u, rsig_col, out_ap, mask_col):
            """out = elu(rsig*u)+1 = exp(min(rsig*u,0)) + max(rsig*u,0)."""
            tmin = pool.tile([128, 512], BF16, tag="tmin")
            nc.vector.tensor_scalar(
                out=tmin[:], in0=u[:], scalar1=rsig_col, scalar2=0.0,
                op0=ALU.mult, op1=ALU.min)
            e = pool.tile([128, 512], BF16, tag="e")
            nc.scalar.activation(out=e[:], in_=tmin[:], func=AF.Exp)
            r = pool.tile([128, 512], BF16, tag="r")
            nc.vector.tensor_scalar(
                out=r[:], in0=u[:], scalar1=rsig_col, scalar2=0.0,
                op0=ALU.mult, op1=ALU.max)
            if mask_col is None:
                nc.vector.tensor_tensor(out=out_ap, in0=e[:], in1=r[:],
                                        op=ALU.add)
            else:
                phi = pool.tile([128, 512], BF16, tag="phi_t")
                nc.vector.tensor_tensor(out=phi[:], in0=e[:], in1=r[:],
                                        op=ALU.add)
                nc.vector.tensor_scalar_mul(out=out_ap, in0=phi[:],
                                            scalar1=mask_col)

        # ---------------- phase A: K/V + kv state ----------------
        with (
            tc.tile_pool(name="kv_ps", bufs=1, space="PSUM") as kv_psp,
            tc.tile_pool(name="proj_psA", bufs=4, space="PSUM") as proj_ps,
            tc.tile_pool(name="sA", bufs=3) as sA,
            tc.tile_pool(name="stA", bufs=4) as stA,
        ):
            # 8 head-pairs packed 3+3+2 per PSUM bank
            kv_ps = [kv_psp.tile([128, n, 130], F32, tag=f"kv{t}",
                                 name=f"kv{t}")
                     for t, n in ((0, 3), (1, 3), (2, 2))]
            for mi in range(NMACRO):
                r0 = mi * 256
                k_nat = sA.tile([128, 2, HS], BF16, tag="k_nat")
                nc.gpsimd.dma_start(
                    out=k_nat[:],
                    in_=kx_d[r0:r0 + 256, :].rearrange("(a p) n -> p a n",
                                                       p=128))
                v_nat = sA.tile([128, 2, HS], BF16, tag="v_nat")
                nc.gpsimd.dma_start(
                    out=v_nat[:],
                    in_=vx_d[r0:r0 + 256, :].rearrange("(a p) n -> p a n",
                                                       p=128))
                kT = sA.tile([128, 2, 8, 128], BF16, tag="kT")
                vT = sA.tile([128, 2, 8, 128], BF16, tag="vT")
                for a in range(2):
                    nc.sync.dma_start_transpose(
                        out=kT[:, a, :, :], in_=k_nat[:, a, :])
                    nc.sync.dma_start_transpose(
                        out=vT[:, a, :, :], in_=v_nat[:, a, :])
                mask_cols = None
                if not mask_trivial:
                    mask_cols = stA.tile([128, 2], F32, tag="mcol")
                    nc.gpsimd.dma_start(
                        out=mask_cols[:],
                        in_=mask_d[r0:r0 + 256, :].rearrange(
                            "(a p) u -> p (a u)", p=128))

                # stats for k_a, k_b, v_a, v_b  (cols 0,1,2,3)
                sx = stA.tile([128, 4], F32, tag="sx")
                sx2 = stA.tile([128, 4], F32, tag="sx2")
                for j, (xn, a) in enumerate(
                        ((k_nat, 0), (k_nat, 1), (v_nat, 0), (v_nat, 1))):
                    _stats(nc, sA, stA, xn[:, a, :], sx, sx2, j)
                negmu = stA.tile([128, 4], F32, tag="negmu")
                nc.vector.tensor_scalar_mul(out=negmu[:], in0=sx[:],
                                            scalar1=-RH)
                musq = stA.tile([128, 4], F32, tag="musq")
                nc.vector.tensor_tensor(out=musq[:], in0=negmu[:],
                                        in1=negmu[:], op=ALU.mult)
                veps = stA.tile([128, 4], F32, tag="veps")
                nc.vector.tensor_scalar(out=veps[:], in0=sx2[:], scalar1=RH,
                                        scalar2=LN_EPS, op0=ALU.mult,
                                        op1=ALU.add)
                nc.vector.tensor_tensor(out=veps[:], in0=veps[:], in1=musq[:],
                                        op=ALU.subtract)
                sig = _newton_rsqrt(nc, stA, veps, 4)

                for a in range(2):
                    phi_k = sA.tile([128, HS], BF16, tag="phi_k")
                    for c in range(2):
                        ps = project_chunk(proj_ps, kT[:, a], "k", c)
                        u = ln_correct(sA, ps, "k", c, negmu[:, a:a + 1])
                        mc = None if mask_cols is None else \
                            mask_cols[:, a:a + 1]
                        elu_phi(sA, u, sig[:, a:a + 1],
                                phi_k[:, c * 512:(c + 1) * 512], mc)
                    vh_aug = sA.tile([128, 8, 130], BF16, tag="vh_aug")
                    if USE_STRIDED_MEMSET:
                        nc.vector.memset(vh_aug[:, :, 64:130:65], 1.0)
                    else:
                        nc.vector.memset(vh_aug[:], 1.0)
                    for c in range(2):
                        ps = project_chunk(proj_ps, vT[:, a], "v", c)
                        u = ln_correct(sA, ps, "v", c, negmu[:, 2 + a:3 + a])
                        # vh = rsig*u into strided (pair, s, d) slots
                        if USE_4D_VH:
                            out_ap = vh_aug[:, 4 * c:4 * c + 4, :].rearrange(
                                "p q (s t) -> p q s t", s=2)[:, :, :, 0:64]
                            nc.vector.tensor_scalar_mul(
                                out=out_ap,
                                in0=u[:].rearrange("p (q s t) -> p q s t",
                                                   q=4, s=2),
                                scalar1=sig[:, 2 + a:3 + a])
                        else:
                            u4 = u[:].rearrange("p (q s t) -> p q s t",
                                                q=4, s=2)
                            for sdx in range(2):
                                nc.vector.tensor_scalar_mul(
                                    out=vh_aug[:, 4 * c:4 * c + 4,
                                               65 * sdx:65 * sdx + 64],
                                    in0=u4[:, :, sdx, :],
                                    scalar1=sig[:, 2 + a:3 + a])
                    for m in range(8):
                        t, j = (m // 3, m % 3) if m < 6 else (2, m - 6)
                        nc.tensor.matmul(
                            kv_ps[t][:, j, :],
                            phi_k[:, m * 128:(m + 1) * 128],
                            vh_aug[:, m, :],
                            start=(mi == 0 and a == 0),
                            stop=(mi == NMACRO - 1 and a == 1),
                            skip_group_check=True)

            nc.vector.tensor_copy(out=kv_sb[:, 0:3, :], in_=kv_ps[0][:])
            nc.vector.tensor_copy(out=kv_sb[:, 3:6, :], in_=kv_ps[1][:])
            nc.vector.tensor_copy(out=kv_sb[:, 6:8, :], in_=kv_ps[2][:])

        # ---------------- q prefetch (ahead of collective in SWDGE FIFO) --
        qpre_pool = ctx.enter_context(tc.tile_pool(name="qpre", bufs=1))
        qpre = qpre_pool.tile([128, NMACRO, 2, HS], BF16, tag="qpre")
        for mi in range(NMACRO):
            nc.gpsimd.dma_start(
                out=qpre[:, mi, :, :],
                in_=qx_d[mi * 256:mi * 256 + 256, :].rearrange(
                    "(a p) n -> p a n", p=128))

        # ---------------- all-reduce kv within batch pairs ----------
        cc_in = dram_p.tile([128, 8, 130], F32)
        cc_out = dram_p.tile([128, 8, 130], F32)
        nc.gpsimd.dma_start(out=cc_in[:], in_=kv_sb[:])
        nc.gpsimd.collective_compute(
            "AllReduce", ALU.add, replica_groups=replica_groups,
            ins=[cc_in.opt()], outs=[cc_out.opt()])

        # ---------------- phase Q: projections + phi_q^T stash -------
        phiqT = wpool.tile([128, NT, 8, 128], BF16, tag="phiqT",
                           name="phiqT")
        with (
            tc.tile_pool(name="proj_psQ", bufs=4, space="PSUM") as proj_ps,
            tc.tile_pool(name="sQ", bufs=3) as sQ,
            tc.tile_pool(name="stQ", bufs=4) as stQ,
        ):
            for mi in range(NMACRO):
                r0 = mi * 256
                q_nat = qpre[:, mi]
                qT = sQ.tile([128, 2, 8, 128], BF16, tag="qT")
                for a in range(2):
                    nc.sync.dma_start_transpose(
                        out=qT[:, a, :, :], in_=q_nat[:, a, :])
                sx = stQ.tile([128, 2], F32, tag="sxq")
                sx2 = stQ.tile([128, 2], F32, tag="sx2q")
                for a in range(2):
                    _stats(nc, sQ, stQ, q_nat[:, a, :], sx, sx2, a)
                negmu = stQ.tile([128, 2], F32, tag="negmuq")
                nc.vector.tensor_scalar_mul(out=negmu[:], in0=sx[:],
                                            scalar1=-RH)
                musq = stQ.tile([128, 2], F32, tag="musqq")
                nc.vector.tensor_tensor(out=musq[:], in0=negmu[:],
                                        in1=negmu[:], op=ALU.mult)
                veps = stQ.tile([128, 2], F32, tag="vepsq")
                nc.vector.tensor_scalar(out=veps[:], in0=sx2[:], scalar1=RH,
                                        scalar2=LN_EPS, op0=ALU.mult,
                                        op1=ALU.add)
                nc.vector.tensor_tensor(out=veps[:], in0=veps[:],
                                        in1=musq[:], op=ALU.subtract)
                sig = _newton_rsqrt(nc, stQ, veps, 2)
                for a in range(2):
                    phi_q = sQ.tile([128, HS], BF16, tag="phi_q")
                    for c in range(2):
                        ps = project_chunk(proj_ps, qT[:, a], "q", c)
                        u = ln_correct(sQ, ps, "q", c, negmu[:, a:a + 1])
                        elu_phi(sQ, u, sig[:, a:a + 1],
                                phi_q[:, c * 512:(c + 1) * 512], None)
                    nc.sync.dma_start_transpose(
                        out=phiqT[:, 2 * mi + a, :, :], in_=phi_q[:])

        # kv2 = reduced state, bf16 (cast during DMA load-back)
        kv2 = consts.tile([128, 8, 130], BF16, tag="kv2")
        nc.gpsimd.dma_start(out=kv2[:], in_=cc_out[:])

        # ---------------- phase B: attn + fc + residual --------------
        with (
            tc.tile_pool(name="nd_ps", bufs=2, space="PSUM") as nd_psp,
            tc.tile_pool(name="fc_ps", bufs=2, space="PSUM") as fc_psp,
            tc.tile_pool(name="sB", bufs=3) as sB,
            tc.tile_pool(name="stB", bufs=4) as stB,
        ):
            for i in range(NT):
                r0 = i * 128
                qres = sB.tile([128, HS], BF16, tag="qres")
                nc.gpsimd.dma_start(out=qres[:], in_=qx_d[r0:r0 + 128, :])

                nds = [nd_psp.tile([128, n, 130], F32, tag=f"nd{t}",
                                   name=f"nd{t}")
                       for t, n in ((0, 3), (1, 3), (2, 2))]
                for m in range(8):
                    t, j = (m // 3, m % 3) if m < 6 else (2, m - 6)
                    nc.tensor.matmul(
                        nds[t][:, j, :], phiqT[:, i, m, :], kv2[:, m, :],
                        start=True, stop=True)

                den = stB.tile([128, NH], F32, tag="den")
                for t, n in ((0, 3), (1, 3), (2, 2)):
                    src = nds[t][:].rearrange("p j (s t) -> p (j s) t",
                                              s=2)[:, :, 64:65]
                    nc.vector.tensor_copy(
                        out=den[:, 6 * t:6 * t + 2 * n].unsqueeze(2),
                        in_=src)
                rd = stB.tile([128, NH], F32, tag="rd")
                nc.vector.tensor_scalar_add(out=rd[:], in0=den[:],
                                            scalar1=ATT_EPS)
                nc.vector.reciprocal(out=rd[:], in_=rd[:])

                attn = sB.tile([128, HS], BF16, tag="attn")
                if USE_BCAST_EVAC:
                    for t, n in ((0, 3), (1, 3), (2, 2)):
                        src = nds[t][:].rearrange("p j (s t) -> p (j s) t",
                                                  s=2)[:, :, 0:64]
                        rd_b = rd[:, 6 * t:6 * t + 2 * n].unsqueeze(2) \
                            .to_broadcast([128, 2 * n, 64])
                        dst = attn[:, 384 * t:384 * t + 128 * n].rearrange(
                            "p (h t) -> p h t", t=64)
                        nc.vector.tensor_tensor(out=dst, in0=src, in1=rd_b,
                                                op=ALU.mult)
                else:
                    for nh in range(NH):
                        t, j, sdx = nh // 6, (nh % 6) // 2, nh % 2
                        nc.scalar.activation(
                            out=attn[:, nh * 64:(nh + 1) * 64],
                            in_=nds[t][:, j, 65 * sdx:65 * sdx + 64],
                            func=AF.Copy, bias=0.0,
                            scale=rd[:, nh:nh + 1])
                attnT = sB.tile([128, 8, 128], BF16, tag="attnT")
                nc.sync.dma_start_transpose(out=attnT[:], in_=attn[:])

                out_sb = sB.tile([128, HS], BF16, tag="out_sb")
                for c in range(2):
                    ps = fc_psp.tile([128, 512], F32, tag="fcp")
                    for kt in range(8):
                        nc.tensor.matmul(
                            ps[:], attnT[:, kt, :],
                            w_sb["fc"][:, kt, c * 512:(c + 1) * 512],
                            start=(kt == 0), stop=(kt == 7))
                    if c_bc["fc"] is not None:
                        nc.vector.tensor_tensor(
                            out=ps[:], in0=ps[:],
                            in1=c_bc["fc"][:, c * 512:(c + 1) * 512],
                            op=ALU.add)
                    nc.vector.tensor_tensor(
                        out=out_sb[:, c * 512:(c + 1) * 512], in0=ps[:],
                        in1=qres[:, c * 512:(c + 1) * 512], op=ALU.add)
                nc.gpsimd.dma_start(out=out_d[r0:r0 + 128, :], in_=out_sb[:])

    nc.compile()
    return nc


_BUILD_CACHE = {}


def _get_nc(flags, replica_groups):
    key = (flags, tuple(tuple(g) for g in replica_groups))
    if key not in _BUILD_CACHE:
        _BUILD_CACHE[key] = build(*flags, replica_groups)
    return _BUILD_CACHE[key]


def make_in_maps(q, k, v, ln_q_g, ln_q_b, wq, bq, ln_k_g, ln_k_b, wk, bk,
                 ln_v_g, ln_v_b, wv, bv, fc_w, fc_b, mask):
    """Host-side prep: bf16 conversion, g folded into W, colsum rows,
    bias rows.  Returns (flags, in_maps)."""
    bf = ml_dtypes.bfloat16
    q = np.ascontiguousarray(q, np.float32).reshape(B * S, HS)
    k = np.ascontiguousarray(k, np.float32).reshape(B * S, HS)
    v = np.ascontiguousarray(v, np.float32).reshape(B * S, HS)
    mask_f = np.ascontiguousarray(mask, np.float32).reshape(B * S, 1)

    ws = {}
    ss = {}
    for name, w, g in (("q", wq, ln_q_g), ("k", wk, ln_k_g),
                       ("v", wv, ln_v_g)):
        wg = np.asarray(w, np.float32) * np.asarray(g, np.float32)[:, None]
        ws[name] = np.ascontiguousarray(wg.astype(bf))
        ss[name] = np.ascontiguousarray(wg.sum(0, dtype=np.float32)
                                        .reshape(1, HS).astype(bf))
    ws["fc"] = np.ascontiguousarray(np.asarray(fc_w, np.float32).astype(bf))

    # c rows: LN bias b enters as +b@W(g-folded) + proj bias
    cs = {}
    for name, bvec, pb in (("q", ln_q_b, bq), ("k", ln_k_b, bk),
                           ("v", ln_v_b, bv)):
        c = (np.asarray(bvec, np.float32) @
             (np.asarray(ws[name], np.float32))) + np.asarray(pb, np.float32)
        cs[name] = np.ascontiguousarray(c.reshape(1, HS), np.float32)
    cs["fc"] = np.ascontiguousarray(
        np.asarray(fc_b, np.float32).reshape(1, HS))

    c_trivial = all(np.all(np.asarray(x) == 0.0) for x in
                    (ln_q_b, ln_k_b, ln_v_b, bq, bk, bv, fc_b))
    mask_trivial = bool(np.all(mask_f == 1.0))

    qb = np.ascontiguousarray(q.astype(bf))
    kb = np.ascontiguousarray(k.astype(bf))
    vb = np.ascontiguousarray(v.astype(bf))

    in_maps = []
    for c in range(NCORES):
        r0, r1 = c * TOK, (c + 1) * TOK
        m = {
            "qx": qb[r0:r1], "kx": kb[r0:r1], "vx": vb[r0:r1],
            "w_q": ws["q"], "w_k": ws["k"], "w_v": ws["v"],
            "w_fc": ws["fc"],
            "s_q": ss["q"], "s_k": ss["k"], "s_v": ss["v"],
        }
        if not c_trivial:
            m.update({f"c_{s}": cs[s] for s in ("q", "k", "v", "fc")})
        if not mask_trivial:
            m["maskx"] = mask_f[r0:r1]
        in_maps.append(m)
    return (c_trivial, mask_trivial), in_maps


def kernel(q, k, v, ln_q_g, ln_q_b, wq, bq, ln_k_g, ln_k_b, wk, bk,
           ln_v_g, ln_v_b, wv, bv, fc_w, fc_b, mask):
    flags, in_maps = make_in_maps(
        q, k, v, ln_q_g, ln_q_b, wq, bq, ln_k_g, ln_k_b, wk, bk,
        ln_v_g, ln_v_b, wv, bv, fc_w, fc_b, mask)
    groups = [[0, 1], [2, 3], [4, 5], [6, 7]]
    nc = _get_nc(flags, groups)
    res = run_bass_kernel_spmd(nc, in_maps, list(range(NCORES)))
    out = np.concatenate(
        [np.asarray(res.results[c]["out"]) for c in range(NCORES)], 0)
    return out.reshape(B, S, HS).astype(np.float32)
